# revision 2
# baseline (speedup 1.0000x reference)
"""DivergentAttention Trainium2 kernel (8 NeuronCores, Bass/Tile), v2.

Problem: GPT-2 style causal self-attention (B=2, S=2048, D=1024, H=16,
hd=64) where heads 0/1/2 re-weight their attention toward a token region
(first/middle/last third of the sequence) with factor 1.6 and renormalize.

Identity: softmax(s)*m / sum(softmax(s)*m) == softmax(s + log m): the region
reweight folds into an additive per-(head, key) bias on the scores. Scores
are small (|s|<~5) so the max-subtraction pass is skipped.

Sharding: core c handles batch c//4 and heads [4*(c%4), 4*(c%4)+4); host
sums the 8 c_proj partials and adds c_proj_b.

v2 layout (all bf16 on the PE):
  - QKV as in v1 (contraction-outer, 8 resident PSUM groups), but the
    PSUM->SBUF copies moved to DVE (tensor_scalar_add with per-partition
    bias), freeing ACT for the exp stream.
  - scoresT [sk=128, q-width<=1024] pieces; exp on ACT (bias log-mult,
    scale 1/8) -> attnT bf16; causal diagonal handled by a 0/1 mask
    multiply on GPSIMD after the exp.
  - AV is FLIPPED: out[q-tile 128, 65] = attnT_tile.T @ [v | 1]: the cost
    model charges moving columns only, so 65 cols/tile-pair halves the AV
    cost vs the [65, width] orientation, and the softmax denominator
    (col 64) lands on the same partitions as q -> normalization is one
    per-partition reciprocal + tensor_scalar_mul on DVE. Per-(head,q-tile)
    accumulator groups are packed 7-per-PSUM-bank (65*4B stride) with
    skip_group_check; hw zeroes only written bytes on start.
  - ao[q, hd] is DMA-xbar-transposed (dma_start_transpose, 14ns/tile) to
    aoT[hd, q] per (q-tile, head-pair) for c_proj, which is interleaved
    into the tail of phase 2 (gated on the last head's drains).
  - out stores are bf16 straight from a GPSIMD PSUM->SBUF copy; host
    upcasts, sums partials, adds c_proj_b.
"""

import numpy as np

import concourse.bass as bass
import concourse.tile as tile
from concourse import mybir
from concourse import bass_utils, bass2jax

# ---------------------------------------------------------------- constants
B, S, D, H, HD = 2, 2048, 1024, 16, 64
NCORES = 8
HPC = 4              # heads per core
GROUPS = 4           # head groups
FOCUS = 1.6
HEAD_REGION = {0: 0, 1: 1, 2: 2}
BF = mybir.dt.bfloat16
F32 = mybir.dt.float32
NT = S // 128         # 16
KO = D // 128         # 8
BATCHES = ((0, 7), (7, 14), (14, 16))   # q-tile batches per PSUM bank

# ------------------------------------------------- walrus multi-wait fixup
# This container's walrus accepts only ONE sync-wait per TPB instruction,
# but Tile attaches one wait per dependency proc. Rewrite the BIR JSON just
# before walrus: hoist all-but-one wait of a multi-wait instruction onto
# standalone same-engine NoOps inserted immediately before it (same-engine
# program order is preserved, so semantics are unchanged).
try:
    import orjson as _json
except ImportError:  # pragma: no cover
    import json as _json

_orig_compile_bir_kernel = bass_utils.compile_bir_kernel
_wfix_counter = [0]


def _fix_bir(bir_json):
    d = _json.loads(bir_json)
    changed = False
    for fn in d.get("functions", []):
        for blk in fn.get("blocks", []):
            out = []
            for inst in blk.get("instructions", []):
                si = inst.get("sync_info")
                if si:
                    waits = si.get("on_wait") or []
                    if len(waits) > 1:
                        changed = True
                        for w in waits[:-1]:
                            _wfix_counter[0] += 1
                            nop = {
                                "engine": inst["engine"],
                                "ins": [],
                                "name": f"I-wfix-{_wfix_counter[0]}",
                                "opcode": "NoOp",
                                "outs": [],
                                "sync_info": {"on_update": [], "on_wait": [w]},
                            }
                            if "debug" in inst:
                                nop["debug"] = inst["debug"]
                            out.append(nop)
                        si["on_wait"] = waits[-1:]
                out.append(inst)
            blk["instructions"] = out
    return _json.dumps(d) if changed else bir_json


def _patched_compile_bir_kernel(bir_json, tmpdir, neff_name="file.neff"):
    return _orig_compile_bir_kernel(_fix_bir(bir_json), tmpdir, neff_name=neff_name)


def _install_waitfix():
    bass_utils.compile_bir_kernel = _patched_compile_bir_kernel
    bass2jax.compile_bir_kernel = _patched_compile_bir_kernel


_install_waitfix()

# ---------------------------------------------------------------- program


def build_program():
    """One SPMD Bass program; per-core differences come in via inputs."""
    nc = bass.Bass()

    hiddenT = nc.dram_tensor("hiddenT", [D, S], BF, kind="ExternalInput")
    w_qkv = nc.dram_tensor("w_qkv", [D, 768], BF, kind="ExternalInput")
    bqk = nc.dram_tensor("bqk", [128, 4], F32, kind="ExternalInput")
    bv_rep = nc.dram_tensor("bv_rep", [128, 256], F32, kind="ExternalInput")
    projw = nc.dram_tensor("projw", [128, 2, D], BF, kind="ExternalInput")
    diag_mask = nc.dram_tensor("diag_mask", [128, 128], BF, kind="ExternalInput")
    logmult = nc.dram_tensor("logmult", [128, HPC, NT], F32, kind="ExternalInput")
    out = nc.dram_tensor("out", [S, D], BF, kind="ExternalOutput")

    with tile.TileContext(nc) as tc:
        with tc.tile_pool(name="persist", bufs=1) as persist:
            qk_sb = persist.tile([128, 4, S], BF)            # 2 MB
            v_sb = persist.tile([128, NT, HPC, 65], BF)      # ~1.06 MB
            ao_sb = persist.tile([128, NT, 2, 128], BF)      # 1 MB
            aoT_sb = persist.tile([128, NT, 2, 128], BF)     # 1 MB
            bqk_sb = persist.tile([128, 4], F32)
            bv_sb = persist.tile([128, 256], F32)
            pw_sb = persist.tile([128, 2, D], BF)
            dm_sb = persist.tile([128, 128], BF)
            lm_sb = persist.tile([128, HPC, NT], F32)

            nc.sync.dma_start(bqk_sb, bqk[:, :])
            nc.vector.memset(v_sb[:, :, :, 64:65], 1.0)

            # ================= phase 1: QKV projection =================
            with tc.tile_pool(name="p1sb", bufs=1) as p1sb, \
                 tc.tile_pool(name="p1ps", bufs=8, space="PSUM") as p1ps:
                hT = p1sb.tile([128, KO, S], BF)              # 4 MB
                w_sb = p1sb.tile([128, KO, 768], BF)          # 1.5 MB
                hT_src = hiddenT.rearrange("(ko p) s -> p ko s", p=128)
                w_src = w_qkv.rearrange("(ko p) n -> p ko n", p=128)
                for ko in range(KO):
                    q = nc.sync if ko % 2 == 0 else nc.scalar
                    q.dma_start(hT[:, ko, :], hT_src[:, ko, :])
                    nc.gpsimd.dma_start(w_sb[:, ko, :], w_src[:, ko, :])
                    if ko == 0:
                        nc.sync.dma_start(bv_sb, bv_rep[:, :])
                        nc.gpsimd.dma_start(dm_sb, diag_mask[:, :])
                        nc.gpsimd.dma_start(lm_sb, logmult[:, :, :])
                        nc.gpsimd.dma_start(pw_sb, projw[:, :, :])

                # qT/kT: out[n-tile, s] = w.T @ hiddenT; rounds: (q01,k01)
                # then (q23,k23) so head 0/1 attention can start early.
                for rnd in range(2):
                    ps8 = [p1ps.tile([128, 512], F32, tag="g", name=f"q{rnd}{i}")
                           for i in range(8)]
                    for ko in range(KO):
                        for i in range(8):
                            nt, sc = (0, 2, 1, 3)[2 * rnd + i // 4], i % 4
                            nc.tensor.matmul(
                                ps8[i],
                                w_sb[:, ko, 128 * nt:128 * nt + 128],
                                hT[:, ko, 512 * sc:512 * sc + 512],
                                start=(ko == 0), stop=(ko == KO - 1),
                            )
                    for i in range(8):
                        nt, sc = (0, 2, 1, 3)[2 * rnd + i // 4], i % 4
                        nc.vector.tensor_scalar_add(
                            qk_sb[:, nt, 512 * sc:512 * sc + 512],
                            ps8[i],
                            bqk_sb[:, nt:nt + 1],
                        )

                # v natural: out[s-tile, (h,hd)] = hidden @ wv; low-slot
                # psum reuse so 4 banks free early for phase-2 pools.
                for rnd in range(8):
                    ps2 = [p1ps.tile([128, 512], F32, tag="g", name=f"v{rnd}{i}")
                           for i in range(2)]
                    for ko in range(KO):
                        for i in range(2):
                            st = 2 * rnd + i
                            nc.tensor.matmul(
                                ps2[i][:, 0:256],
                                hT[:, ko, 128 * st:128 * st + 128],
                                w_sb[:, ko, 512:768],
                                start=(ko == 0), stop=(ko == KO - 1),
                            )
                    for i in range(2):
                        st = 2 * rnd + i
                        nc.vector.tensor_add(
                            out=v_sb[:, st, :, 0:64],
                            in0=ps2[i][:, 0:256].rearrange("p (h d) -> p h d", d=64),
                            in1=bv_sb.rearrange("p (h d) -> p h d", d=64),
                        )

            # ============ phase 2: attention + interleaved c_proj ============
            with tc.tile_pool(name="p2at", bufs=6) as p2at, \
                 tc.tile_pool(name="p2rec", bufs=4) as p2rec, \
                 tc.tile_pool(name="p2o", bufs=4) as p2o, \
                 tc.tile_pool(name="p2sc", bufs=2, space="PSUM") as p2sc, \
                 tc.tile_pool(name="p2av", bufs=2, space="PSUM") as p2av, \
                 tc.tile_pool(name="p2cp", bufs=2, space="PSUM") as p2cp:

                av_tiles = {}

                def get_av(lh, b):
                    if (lh, b) not in av_tiles:
                        pool = p2av if b < 2 else p2cp
                        tag = "av" if b < 2 else "cp"
                        av_tiles[(lh, b)] = pool.tile(
                            [128, 512], F32, tag=tag, name=f"av{lh}{b}")
                    return av_tiles[(lh, b)]

                def drain(lh, b):
                    # all q-tiles of this batch are fully accumulated:
                    # reciprocal of the denominator column, then one
                    # per-partition scale per q-tile into ao_sb (bf16).
                    t0, t1 = BATCHES[b]
                    nb = t1 - t0
                    av = av_tiles.pop((lh, b))
                    j, hp = lh // 2, lh % 2
                    rec = p2rec.tile([128, 8], F32, tag="rec")
                    den = av[:, 0:65 * nb].rearrange(
                        "p (n c) -> p n c", c=65)[:, :, 64:65]
                    nc.vector.reciprocal(rec[:, 0:nb], den)
                    for k in range(nb):
                        tau = t0 + k
                        nc.vector.tensor_scalar_mul(
                            ao_sb[:, tau, j, 64 * hp:64 * hp + 64],
                            av[:, 65 * k:65 * k + 64],
                            rec[:, k:k + 1],
                        )
                    if hp == 1:
                        # both heads of pair j have drained these q-tiles:
                        # transpose ao[q, hd] -> aoT[hd, q] on the DMA xbar.
                        for k in range(nb):
                            tau = t0 + k
                            nc.sync.dma_start_transpose(
                                aoT_sb[:, tau, j, :], ao_sb[:, tau, j, :])
                        if lh == 3:
                            for k in range(nb):
                                cproj(t0 + k)

                def cproj(tau):
                    for ec in range(2):
                        ps = p2cp.tile([128, 512], F32, tag="cp",
                                       name=f"pr{tau}{ec}")
                        for j in range(2):
                            nc.tensor.matmul(
                                ps,
                                aoT_sb[:, tau, j, :],
                                pw_sb[:, j, 512 * ec:512 * ec + 512],
                                start=(j == 0), stop=(j == 1),
                            )
                        o_sb = p2o.tile([128, 512], BF, tag="o")
                        nc.gpsimd.tensor_copy(o_sb, ps)
                        oq = nc.sync if ec == 0 else nc.gpsimd
                        oq.dma_start(
                            out[128 * tau:128 * tau + 128,
                                512 * ec:512 * ec + 512],
                            o_sb,
                        )

                def tail(lh, t, q0, width, at_sb):
                    # exp consumers: causal 0/1 mask on the diagonal block
                    # (GPSIMD, all-SBUF) + flipped AV accumulation.
                    if q0 == 128 * t:
                        nc.gpsimd.tensor_mul(
                            out=at_sb[:, 0:128], in0=at_sb[:, 0:128],
                            in1=dm_sb,
                        )
                    v_aug = v_sb[:, t, lh, :]
                    for tau in range(q0 // 128, (q0 + width) // 128):
                        b = 0 if tau < 7 else (1 if tau < 14 else 2)
                        av = get_av(lh, b)
                        col = 65 * (tau - BATCHES[b][0])
                        off = 128 * tau - q0
                        nc.tensor.matmul(
                            av[:, col:col + 65],
                            at_sb[:, off:off + 128],
                            v_aug,
                            start=(t == 0), stop=(t == tau),
                            skip_group_check=True,
                        )
                    if q0 + width == 1024 and t == 6:
                        drain(lh, 0)
                    elif q0 + width == 2048 and t == 13:
                        drain(lh, 1)
                    elif q0 + width == 2048 and t == 15:
                        drain(lh, 2)

                pieces = []
                for lh in range(HPC):
                    for t in range(NT):
                        for hf in range(2):
                            q0 = max(128 * t, 1024 * hf)
                            q1 = 1024 * (hf + 1)
                            if q0 < q1:
                                pieces.append((lh, t, q0, q1 - q0))

                pending = []
                for lh, t, q0, width in pieces:
                    bp = 64 * (lh % 2)
                    q_nt = lh // 2
                    k_nt = 2 + lh // 2
                    lhsT_k = qk_sb[bp:bp + 64, k_nt, 128 * t:128 * t + 128]
                    sc_ps = p2sc.tile([128, 1024], F32, tag="sc")
                    off = 0
                    while off < width:
                        w512 = min(512, width - off)
                        nc.tensor.matmul(
                            sc_ps[:, off:off + w512],
                            lhsT_k,
                            qk_sb[bp:bp + 64, q_nt, q0 + off:q0 + off + w512],
                            start=True, stop=True,
                        )
                        off += w512
                    at_sb = p2at.tile([128, 1024], BF, tag="attnT")
                    nc.scalar.activation(
                        at_sb[:, :width], sc_ps[:, :width],
                        mybir.ActivationFunctionType.Exp,
                        bias=lm_sb[:, lh, t:t + 1], scale=0.125,
                    )
                    pending.append((lh, t, q0, width, at_sb))
                    if len(pending) > 4:
                        tail(*pending.pop(0))
                for pc in pending:
                    tail(*pc)
    return nc


_NC = None


def _get_nc():
    global _NC
    if _NC is None:
        _NC = build_program()
    return _NC


# ---------------------------------------------------------------- host prep

def make_in_maps(hidden_states, c_attn_w, c_attn_b, c_proj_w):
    import ml_dtypes
    bf16 = ml_dtypes.bfloat16

    first_end = S // 3
    second_end = 2 * S // 3
    pos = np.arange(S)
    regions = [pos < first_end,
               (pos >= first_end) & (pos < second_end),
               pos >= second_end]
    mult = np.ones((H, S), dtype=np.float64)
    for h, r in HEAD_REGION.items():
        mult[h] = 1.0 + (FOCUS - 1.0) * regions[r].astype(np.float64)
    logm = np.log(mult).astype(np.float32)  # [H, S]

    p = np.arange(128)[:, None]
    j = np.arange(128)[None, :]
    diag = (j >= p).astype(np.float32)  # 0/1 keep-mask, applied post-exp

    in_maps = []
    for c in range(NCORES):
        b, g = divmod(c, GROUPS)
        h0 = HPC * g
        cs = slice(256 * g, 256 * g + 256)
        w_qkv = np.concatenate(
            [c_attn_w[:, cs], c_attn_w[:, 1024:2048][:, cs],
             c_attn_w[:, 2048:3072][:, cs]], axis=1,
        )
        bqk = np.concatenate(
            [c_attn_b[cs], c_attn_b[1024:2048][cs]]
        ).reshape(4, 128).T.copy().astype(np.float32)
        bv = np.broadcast_to(
            c_attn_b[2048:3072][cs], (128, 256)
        ).astype(np.float32).copy()
        # pw2[p, j, e]: head pair j=(2j, 2j+1); p<64 -> head 2j row p,
        # p>=64 -> head 2j+1 row p-64  (matches aoT partition layout)
        pw = c_proj_w[64 * h0:64 * h0 + 256, :].reshape(2, 128, D)
        pw = np.ascontiguousarray(pw.transpose(1, 0, 2))
        lm = logm[h0:h0 + HPC].reshape(HPC, S // 128, 128)
        lm = np.ascontiguousarray(lm.transpose(2, 0, 1)).astype(np.float32)
        in_maps.append({
            "hiddenT": np.ascontiguousarray(hidden_states[b].T).astype(bf16),
            "w_qkv": np.ascontiguousarray(w_qkv).astype(bf16),
            "bqk": bqk,
            "bv_rep": bv,
            "projw": pw.astype(bf16),
            "diag_mask": diag.astype(bf16),
            "logmult": lm,
        })
    return in_maps


def run_cores(in_maps, trace=False, **kw):
    from concourse.bass_utils import run_bass_kernel_spmd
    nc = _get_nc()
    return run_bass_kernel_spmd(nc, in_maps, core_ids=list(range(NCORES)),
                                trace=trace, **kw)


def kernel(hidden_states, c_attn_w, c_attn_b, c_proj_w, c_proj_b):
    hidden_states = np.asarray(hidden_states, dtype=np.float32)
    c_attn_w = np.asarray(c_attn_w, dtype=np.float32)
    c_attn_b = np.asarray(c_attn_b, dtype=np.float32)
    c_proj_w = np.asarray(c_proj_w, dtype=np.float32)
    c_proj_b = np.asarray(c_proj_b, dtype=np.float32)

    in_maps = make_in_maps(hidden_states, c_attn_w, c_attn_b, c_proj_w)
    res = run_cores(in_maps)
    out = np.zeros((B, S, D), dtype=np.float32)
    for c in range(NCORES):
        out[c // GROUPS] += np.asarray(res.results[c]["out"], dtype=np.float32)
    out += c_proj_b[None, None, :]
    return out


# revision 3
# speedup vs baseline: 1.1628x; 1.1628x over previous
"""DivergentAttention Trainium2 kernel (8 NeuronCores, Bass/Tile), v2.1.

Problem: GPT-2 style causal self-attention (B=2, S=2048, D=1024, H=16,
hd=64) where heads 0/1/2 re-weight their attention toward a token region
(first/middle/last third of the sequence) with factor 1.6 and renormalize.

Identity: softmax(s)*m / sum(softmax(s)*m) == softmax(s + log m): the region
reweight folds into an additive per-(head, key) bias on the scores. Scores
are small (|s|<~5) so the max-subtraction pass is skipped.

Sharding: core c handles batch c//4 and heads [4*(c%4), 4*(c%4)+4); host
sums the 8 c_proj partials and adds c_proj_b.

v2 design (all bf16 on the PE; cost model charges matmuls by moving
columns only):
  - AV is FLIPPED: out[q-tile 128, 65] = attnT_tile.T @ [v | 1]: 65 moving
    cols per (q,k) tile pair halves AV cost vs the [65, q-width]
    orientation, and the softmax denominator (col 64) lands on the same
    partitions as q, so normalization is a per-partition reciprocal +
    tensor_scalar_mul on DVE (no partition broadcast needed).
  - Per-(head,q-tile) PSUM accumulator groups are packed 7-per-bank at
    65*4B stride with skip_group_check; hw zeroes only written bytes on
    start_tensor_calc.
  - ao[q, hd] is DMA-xbar-transposed to aoT[hd, q] per (q-tile, head-pair)
    for c_proj, which is interleaved into the tail of phase 2.
  - ACT runs ONLY the exp stream; all PSUM->SBUF copies are on DVE
    (with per-partition bias via tensor_scalar_add) or GPSIMD.
  - Emission interleaves phase-1 QKV rounds with attention pieces so the
    exp stream starts ~10us in (engine queues drain in program order, so
    program order is priority): qk rounds are 4x4 resident PSUM groups,
    v projection runs 1 group at a time under the attention stream.
"""

import numpy as np

import concourse.bass as bass
import concourse.tile as tile
from concourse import mybir
from concourse import bass_utils, bass2jax

# ---------------------------------------------------------------- constants
B, S, D, H, HD = 2, 2048, 1024, 16, 64
NCORES = 8
HPC = 4              # heads per core
GROUPS = 4           # head groups
FOCUS = 1.6
HEAD_REGION = {0: 0, 1: 1, 2: 2}
BF = mybir.dt.bfloat16
F32 = mybir.dt.float32
NT = S // 128         # 16
KO = D // 128         # 8
BATCHES = ((0, 7), (7, 14), (14, 16))   # q-tile batches per PSUM bank

# ------------------------------------------------- walrus multi-wait fixup
# This container's walrus accepts only ONE sync-wait per TPB instruction,
# but Tile attaches one wait per dependency proc. Rewrite the BIR JSON just
# before walrus: hoist all-but-one wait of a multi-wait instruction onto
# standalone same-engine NoOps inserted immediately before it (same-engine
# program order is preserved, so semantics are unchanged).
try:
    import orjson as _json
except ImportError:  # pragma: no cover
    import json as _json

_orig_compile_bir_kernel = bass_utils.compile_bir_kernel
_wfix_counter = [0]


def _fix_bir(bir_json):
    d = _json.loads(bir_json)
    changed = False
    for fn in d.get("functions", []):
        for blk in fn.get("blocks", []):
            out = []
            for inst in blk.get("instructions", []):
                si = inst.get("sync_info")
                if si:
                    waits = si.get("on_wait") or []
                    if len(waits) > 1:
                        changed = True
                        for w in waits[:-1]:
                            _wfix_counter[0] += 1
                            nop = {
                                "engine": inst["engine"],
                                "ins": [],
                                "name": f"I-wfix-{_wfix_counter[0]}",
                                "opcode": "NoOp",
                                "outs": [],
                                "sync_info": {"on_update": [], "on_wait": [w]},
                            }
                            if "debug" in inst:
                                nop["debug"] = inst["debug"]
                            out.append(nop)
                        si["on_wait"] = waits[-1:]
                out.append(inst)
            blk["instructions"] = out
    return _json.dumps(d) if changed else bir_json


def _patched_compile_bir_kernel(bir_json, tmpdir, neff_name="file.neff"):
    return _orig_compile_bir_kernel(_fix_bir(bir_json), tmpdir, neff_name=neff_name)


def _install_waitfix():
    bass_utils.compile_bir_kernel = _patched_compile_bir_kernel
    bass2jax.compile_bir_kernel = _patched_compile_bir_kernel


_install_waitfix()

# ---------------------------------------------------------------- program


def build_program():
    """One SPMD Bass program; per-core differences come in via inputs."""
    nc = bass.Bass()

    hiddenT = nc.dram_tensor("hiddenT", [D, S], BF, kind="ExternalInput")
    w_qkv = nc.dram_tensor("w_qkv", [D, 768], BF, kind="ExternalInput")
    bqk = nc.dram_tensor("bqk", [128, 4], F32, kind="ExternalInput")
    bv_rep = nc.dram_tensor("bv_rep", [128, 256], F32, kind="ExternalInput")
    projw = nc.dram_tensor("projw", [128, 2, D], BF, kind="ExternalInput")
    diag_mask = nc.dram_tensor("diag_mask", [128, 128], BF, kind="ExternalInput")
    logmult = nc.dram_tensor("logmult", [128, HPC, NT], F32, kind="ExternalInput")
    out = nc.dram_tensor("out", [S, D], BF, kind="ExternalOutput")

    with tile.TileContext(nc) as tc:
        with tc.tile_pool(name="persist", bufs=1) as persist, \
             tc.tile_pool(name="p2at", bufs=16) as p2at, \
             tc.tile_pool(name="p2rec", bufs=4) as p2rec, \
             tc.tile_pool(name="p2o", bufs=4) as p2o, \
             tc.tile_pool(name="p2sc", bufs=2, space="PSUM") as p2sc, \
             tc.tile_pool(name="mix", bufs=4, space="PSUM") as mix:

            qk_sb = persist.tile([128, 4, S], BF)            # 2 MB
            v_sb = persist.tile([128, NT, HPC, 65], BF)      # ~1.06 MB
            ao_sb = persist.tile([128, NT, 2, 128], BF)      # 1 MB
            aoT_sb = persist.tile([128, NT, 2, 128], BF)     # 1 MB
            bqk_sb = persist.tile([128, 4], F32)
            bv_sb = persist.tile([128, 256], F32)
            pw_sb = persist.tile([128, 2, D], BF)
            dm_sb = persist.tile([128, 128], BF)
            lm_sb = persist.tile([128, HPC, NT], F32)
            hT = persist.tile([128, KO, S], BF)              # 4 MB
            w_sb = persist.tile([128, KO, 768], BF)          # 1.5 MB

            nc.vector.memset(v_sb[:, :, :, 64:65], 1.0)

            # ---- loads ----
            hT_src = hiddenT.rearrange("(ko p) s -> p ko s", p=128)
            w_src = w_qkv.rearrange("(ko p) n -> p ko n", p=128)
            for ko in range(KO):
                q = nc.sync if ko % 2 == 0 else nc.scalar
                q.dma_start(hT[:, ko, :], hT_src[:, ko, :])
                nc.gpsimd.dma_start(w_sb[:, ko, :], w_src[:, ko, :])
                if ko == 0:
                    nc.sync.dma_start(bqk_sb, bqk[:, :])
                    nc.sync.dma_start(bv_sb, bv_rep[:, :])
                    nc.gpsimd.dma_start(dm_sb, diag_mask[:, :])
                    nc.gpsimd.dma_start(lm_sb, logmult[:, :, :])
                    nc.gpsimd.dma_start(pw_sb, projw[:, :, :])

            # ---- phase-1 building blocks ----
            def qk_round(r):
                # 4 resident PSUM groups, contraction-outer: (q, k) n-tile
                # pair for one head pair x two 512-col s-chunks.
                nts = (0, 2) if r < 2 else (1, 3)
                scs = (0, 1) if r % 2 == 0 else (2, 3)
                quads = [(nt, sc) for nt in nts for sc in scs]
                ps4 = [mix.tile([128, 512], F32, tag="m", name=f"qk{r}{i}")
                       for i in range(4)]
                for ko in range(KO):
                    for i, (nt, sc) in enumerate(quads):
                        nc.tensor.matmul(
                            ps4[i],
                            w_sb[:, ko, 128 * nt:128 * nt + 128],
                            hT[:, ko, 512 * sc:512 * sc + 512],
                            start=(ko == 0), stop=(ko == KO - 1),
                        )
                for i, (nt, sc) in enumerate(quads):
                    nc.vector.tensor_scalar_add(
                        qk_sb[:, nt, 512 * sc:512 * sc + 512],
                        ps4[i],
                        bqk_sb[:, nt:nt + 1],
                    )

            def v_tile(st):
                ps = mix.tile([128, 512], F32, tag="m", name=f"v{st}")
                for ko in range(KO):
                    nc.tensor.matmul(
                        ps[:, 0:256],
                        hT[:, ko, 128 * st:128 * st + 128],
                        w_sb[:, ko, 512:768],
                        start=(ko == 0), stop=(ko == KO - 1),
                    )
                nc.vector.tensor_add(
                    out=v_sb[:, st, :, 0:64],
                    in0=ps[:, 0:256].rearrange("p (h d) -> p h d", d=64),
                    in1=bv_sb.rearrange("p (h d) -> p h d", d=64),
                )

            # ---- phase-2 building blocks ----
            av_tiles = {}

            def get_av(lh, b):
                if (lh, b) not in av_tiles:
                    av_tiles[(lh, b)] = mix.tile(
                        [128, 512], F32, tag="m", name=f"av{lh}{b}")
                return av_tiles[(lh, b)]

            def cproj(tau):
                for ec in range(2):
                    ps = mix.tile([128, 512], F32, tag="m", name=f"pr{tau}{ec}")
                    for j in range(2):
                        nc.tensor.matmul(
                            ps,
                            aoT_sb[:, tau, j, :],
                            pw_sb[:, j, 512 * ec:512 * ec + 512],
                            start=(j == 0), stop=(j == 1),
                        )
                    o_sb = p2o.tile([128, 512], BF, tag="o")
                    cpq = nc.vector if ec == 0 else nc.gpsimd
                    cpq.tensor_copy(o_sb, ps)
                    oq = nc.sync if ec == 0 else nc.gpsimd
                    oq.dma_start(
                        out[128 * tau:128 * tau + 128,
                            512 * ec:512 * ec + 512],
                        o_sb,
                    )

            def drain(lh, b):
                # all q-tiles of this batch fully accumulated: reciprocal
                # of the denominator column, per-partition scale into ao_sb.
                t0, t1 = BATCHES[b]
                nb = t1 - t0
                av = av_tiles.pop((lh, b))
                j, hp = lh // 2, lh % 2
                rec = p2rec.tile([128, 8], F32, tag="rec")
                den = av[:, 0:65 * nb].rearrange(
                    "p (n c) -> p n c", c=65)[:, :, 64:65]
                nc.vector.reciprocal(rec[:, 0:nb], den)
                for k in range(nb):
                    tau = t0 + k
                    nc.vector.tensor_scalar_mul(
                        ao_sb[:, tau, j, 64 * hp:64 * hp + 64],
                        av[:, 65 * k:65 * k + 64],
                        rec[:, k:k + 1],
                    )
                if hp == 1:
                    # both heads of pair j drained: transpose ao[q, hd] ->
                    # aoT[hd, q] on the DMA xbar; after the last pair,
                    # this q-tile's c_proj is fully unblocked.
                    for k in range(nb):
                        tau = t0 + k
                        nc.sync.dma_start_transpose(
                            aoT_sb[:, tau, j, :], ao_sb[:, tau, j, :])
                    if lh == 3:
                        for k in range(nb):
                            cproj(t0 + k)

            def tail(lh, t, q0, width, at_sb):
                # exp consumers: causal 0/1 mask on the diagonal block
                # (GPSIMD, all-SBUF) + flipped AV accumulation.
                if q0 == 128 * t:
                    nc.gpsimd.tensor_mul(
                        out=at_sb[:, 0:128], in0=at_sb[:, 0:128],
                        in1=dm_sb,
                    )
                v_aug = v_sb[:, t, lh, :]
                for tau in range(q0 // 128, (q0 + width) // 128):
                    b = 0 if tau < 7 else (1 if tau < 14 else 2)
                    av = get_av(lh, b)
                    col = 65 * (tau - BATCHES[b][0])
                    off = 128 * tau - q0
                    nc.tensor.matmul(
                        av[:, col:col + 65],
                        at_sb[:, off:off + 128],
                        v_aug,
                        start=(t == 0), stop=(t == tau),
                        skip_group_check=True,
                    )
                if q0 + width == 1024 and t == 6:
                    drain(lh, 0)
                elif q0 + width == 2048 and t == 13:
                    drain(lh, 1)
                elif q0 + width == 2048 and t == 15:
                    drain(lh, 2)

            pending = []

            def piece(lh, t, hf):
                q0 = max(128 * t, 1024 * hf)
                q1 = 1024 * (hf + 1)
                if q0 >= q1:
                    return
                width = q1 - q0
                bp = 64 * (lh % 2)
                q_nt = lh // 2
                k_nt = 2 + lh // 2
                lhsT_k = qk_sb[bp:bp + 64, k_nt, 128 * t:128 * t + 128]
                sc_ps = p2sc.tile([128, 1024], F32, tag="sc")
                off = 0
                while off < width:
                    w512 = min(512, width - off)
                    nc.tensor.matmul(
                        sc_ps[:, off:off + w512],
                        lhsT_k,
                        qk_sb[bp:bp + 64, q_nt, q0 + off:q0 + off + w512],
                        start=True, stop=True,
                    )
                    off += w512
                at_sb = p2at.tile([128, 1024], BF, tag="attnT")
                nc.scalar.activation(
                    at_sb[:, :width], sc_ps[:, :width],
                    mybir.ActivationFunctionType.Exp,
                    bias=lm_sb[:, lh, t:t + 1], scale=0.125,
                )
                pending.append((lh, t, q0, width, at_sb))
                if len(pending) > 4:
                    tail(*pending.pop(0))

            # ---- interleaved emission: program order is engine priority ----
            qk_round(0)                       # q01/k01 cols 0:1024
            for t in range(8):
                piece(0, t, 0)
            qk_round(1)                       # q01/k01 cols 1024:2048
            for t in range(16):
                piece(0, t, 1)
            for st in range(8):
                v_tile(st)
            for t in range(8):
                piece(1, t, 0)
            for st in range(8, 16):
                v_tile(st)
            for t in range(16):
                piece(1, t, 1)
            qk_round(2)                       # q23/k23 cols 0:1024
            for t in range(8):
                piece(2, t, 0)
            qk_round(3)                       # q23/k23 cols 1024:2048
            for t in range(16):
                piece(2, t, 1)
            for t in range(8):
                piece(3, t, 0)
            for t in range(16):
                piece(3, t, 1)
            for pc in pending:
                tail(*pc)
            pending.clear()
    return nc


_NC = None


def _get_nc():
    global _NC
    if _NC is None:
        _NC = build_program()
    return _NC


# ---------------------------------------------------------------- host prep

def make_in_maps(hidden_states, c_attn_w, c_attn_b, c_proj_w):
    import ml_dtypes
    bf16 = ml_dtypes.bfloat16

    first_end = S // 3
    second_end = 2 * S // 3
    pos = np.arange(S)
    regions = [pos < first_end,
               (pos >= first_end) & (pos < second_end),
               pos >= second_end]
    mult = np.ones((H, S), dtype=np.float64)
    for h, r in HEAD_REGION.items():
        mult[h] = 1.0 + (FOCUS - 1.0) * regions[r].astype(np.float64)
    logm = np.log(mult).astype(np.float32)  # [H, S]

    p = np.arange(128)[:, None]
    j = np.arange(128)[None, :]
    diag = (j >= p).astype(np.float32)  # 0/1 keep-mask, applied post-exp

    in_maps = []
    for c in range(NCORES):
        b, g = divmod(c, GROUPS)
        h0 = HPC * g
        cs = slice(256 * g, 256 * g + 256)
        w_qkv = np.concatenate(
            [c_attn_w[:, cs], c_attn_w[:, 1024:2048][:, cs],
             c_attn_w[:, 2048:3072][:, cs]], axis=1,
        )
        bqk = np.concatenate(
            [c_attn_b[cs], c_attn_b[1024:2048][cs]]
        ).reshape(4, 128).T.copy().astype(np.float32)
        bv = np.broadcast_to(
            c_attn_b[2048:3072][cs], (128, 256)
        ).astype(np.float32).copy()
        # pw2[p, j, e]: head pair j=(2j, 2j+1); p<64 -> head 2j row p,
        # p>=64 -> head 2j+1 row p-64  (matches aoT partition layout)
        pw = c_proj_w[64 * h0:64 * h0 + 256, :].reshape(2, 128, D)
        pw = np.ascontiguousarray(pw.transpose(1, 0, 2))
        lm = logm[h0:h0 + HPC].reshape(HPC, S // 128, 128)
        lm = np.ascontiguousarray(lm.transpose(2, 0, 1)).astype(np.float32)
        in_maps.append({
            "hiddenT": np.ascontiguousarray(hidden_states[b].T).astype(bf16),
            "w_qkv": np.ascontiguousarray(w_qkv).astype(bf16),
            "bqk": bqk,
            "bv_rep": bv,
            "projw": pw.astype(bf16),
            "diag_mask": diag.astype(bf16),
            "logmult": lm,
        })
    return in_maps


def run_cores(in_maps, trace=False, **kw):
    from concourse.bass_utils import run_bass_kernel_spmd
    nc = _get_nc()
    return run_bass_kernel_spmd(nc, in_maps, core_ids=list(range(NCORES)),
                                trace=trace, **kw)


def kernel(hidden_states, c_attn_w, c_attn_b, c_proj_w, c_proj_b):
    hidden_states = np.asarray(hidden_states, dtype=np.float32)
    c_attn_w = np.asarray(c_attn_w, dtype=np.float32)
    c_attn_b = np.asarray(c_attn_b, dtype=np.float32)
    c_proj_w = np.asarray(c_proj_w, dtype=np.float32)
    c_proj_b = np.asarray(c_proj_b, dtype=np.float32)

    in_maps = make_in_maps(hidden_states, c_attn_w, c_attn_b, c_proj_w)
    res = run_cores(in_maps)
    out = np.zeros((B, S, D), dtype=np.float32)
    for c in range(NCORES):
        out[c // GROUPS] += np.asarray(res.results[c]["out"], dtype=np.float32)
    out += c_proj_b[None, None, :]
    return out


# revision 7
# speedup vs baseline: 1.2067x; 1.0377x over previous
"""DivergentAttention Trainium2 kernel (8 NeuronCores, Bass/Tile), v2.1.

Problem: GPT-2 style causal self-attention (B=2, S=2048, D=1024, H=16,
hd=64) where heads 0/1/2 re-weight their attention toward a token region
(first/middle/last third of the sequence) with factor 1.6 and renormalize.

Identity: softmax(s)*m / sum(softmax(s)*m) == softmax(s + log m): the region
reweight folds into an additive per-(head, key) bias on the scores. Scores
are small (|s|<~5) so the max-subtraction pass is skipped.

Sharding: core c handles batch c//4 and heads [4*(c%4), 4*(c%4)+4); host
sums the 8 c_proj partials and adds c_proj_b.

v2 design (all bf16 on the PE; cost model charges matmuls by moving
columns only):
  - AV is FLIPPED: out[q-tile 128, 65] = attnT_tile.T @ [v | 1]: 65 moving
    cols per (q,k) tile pair halves AV cost vs the [65, q-width]
    orientation, and the softmax denominator (col 64) lands on the same
    partitions as q, so normalization is a per-partition reciprocal +
    tensor_scalar_mul on DVE (no partition broadcast needed).
  - Per-(head,q-tile) PSUM accumulator groups are packed 7-per-bank at
    65*4B stride with skip_group_check; hw zeroes only written bytes on
    start_tensor_calc.
  - ao[q, hd] is DMA-xbar-transposed to aoT[hd, q] per (q-tile, head-pair)
    for c_proj, which is interleaved into the tail of phase 2.
  - ACT runs ONLY the exp stream; all PSUM->SBUF copies are on DVE
    (with per-partition bias via tensor_scalar_add) or GPSIMD.
  - Emission interleaves phase-1 QKV rounds with attention pieces so the
    exp stream starts ~10us in (engine queues drain in program order, so
    program order is priority): qk rounds are 4x4 resident PSUM groups,
    v projection runs 1 group at a time under the attention stream.
"""

import numpy as np

import concourse.bass as bass
import concourse.tile as tile
from concourse import mybir
from concourse import bass_utils, bass2jax

# ---------------------------------------------------------------- constants
B, S, D, H, HD = 2, 2048, 1024, 16, 64
NCORES = 8
HPC = 4              # heads per core
GROUPS = 4           # head groups
FOCUS = 1.6
HEAD_REGION = {0: 0, 1: 1, 2: 2}
BF = mybir.dt.bfloat16
F32 = mybir.dt.float32
NT = S // 128         # 16
KO = D // 128         # 8
BATCHES = ((0, 7), (7, 14), (14, 16))   # q-tile batches per PSUM bank

# ------------------------------------------------- walrus multi-wait fixup
# This container's walrus accepts only ONE sync-wait per TPB instruction,
# but Tile attaches one wait per dependency proc. Rewrite the BIR JSON just
# before walrus: hoist all-but-one wait of a multi-wait instruction onto
# standalone same-engine NoOps inserted immediately before it (same-engine
# program order is preserved, so semantics are unchanged).
try:
    import orjson as _json
except ImportError:  # pragma: no cover
    import json as _json

_orig_compile_bir_kernel = bass_utils.compile_bir_kernel
_wfix_counter = [0]


def _fix_bir(bir_json):
    d = _json.loads(bir_json)
    changed = False
    for fn in d.get("functions", []):
        for blk in fn.get("blocks", []):
            out = []
            for inst in blk.get("instructions", []):
                si = inst.get("sync_info")
                if si:
                    waits = si.get("on_wait") or []
                    if len(waits) > 1:
                        changed = True
                        for w in waits[:-1]:
                            _wfix_counter[0] += 1
                            nop = {
                                "engine": inst["engine"],
                                "ins": [],
                                "name": f"I-wfix-{_wfix_counter[0]}",
                                "opcode": "NoOp",
                                "outs": [],
                                "sync_info": {"on_update": [], "on_wait": [w]},
                            }
                            if "debug" in inst:
                                nop["debug"] = inst["debug"]
                            out.append(nop)
                        si["on_wait"] = waits[-1:]
                out.append(inst)
            blk["instructions"] = out
    return _json.dumps(d) if changed else bir_json


def _patched_compile_bir_kernel(bir_json, tmpdir, neff_name="file.neff"):
    return _orig_compile_bir_kernel(_fix_bir(bir_json), tmpdir, neff_name=neff_name)


def _install_waitfix():
    bass_utils.compile_bir_kernel = _patched_compile_bir_kernel
    bass2jax.compile_bir_kernel = _patched_compile_bir_kernel


_install_waitfix()

# ---------------------------------------------------------------- program


def build_program():
    """One SPMD Bass program; per-core differences come in via inputs."""
    nc = bass.Bass()

    hiddenT = nc.dram_tensor("hiddenT", [D, S], BF, kind="ExternalInput")
    w_qkv = nc.dram_tensor("w_qkv", [D, 768], BF, kind="ExternalInput")
    bqk = nc.dram_tensor("bqk", [128, 4], F32, kind="ExternalInput")
    bv_rep = nc.dram_tensor("bv_rep", [128, 256], F32, kind="ExternalInput")
    projw = nc.dram_tensor("projw", [128, 2, D], BF, kind="ExternalInput")
    diag_mask = nc.dram_tensor("diag_mask", [128, 128], BF, kind="ExternalInput")
    logmult = nc.dram_tensor("logmult", [128, HPC, NT], F32, kind="ExternalInput")
    out = nc.dram_tensor("out", [S, D], BF, kind="ExternalOutput")

    with tile.TileContext(nc) as tc:
        with tc.tile_pool(name="persist", bufs=1) as persist, \
             tc.tile_pool(name="p2at", bufs=16) as p2at, \
             tc.tile_pool(name="p2rec", bufs=4) as p2rec, \
             tc.tile_pool(name="p2o", bufs=4) as p2o, \
             tc.tile_pool(name="p2sc", bufs=2, space="PSUM") as p2sc, \
             tc.tile_pool(name="p2av", bufs=2, space="PSUM") as p2av, \
             tc.tile_pool(name="mix", bufs=2, space="PSUM") as mix:

            qk_sb = persist.tile([128, 4, S], BF)            # 2 MB
            v_sb = persist.tile([128, NT, HPC, 65], BF)      # ~1.06 MB
            ao_sb = persist.tile([128, NT, 2, 128], BF)      # 1 MB
            aoT_sb = persist.tile([128, NT, 2, 128], BF)     # 1 MB
            bqk_sb = persist.tile([128, 4], F32)
            bv_sb = persist.tile([128, 256], F32)
            pw_sb = persist.tile([128, 2, D], BF)
            dm_sb = persist.tile([128, 128], BF)
            lm_sb = persist.tile([128, HPC, NT], F32)
            hT = persist.tile([128, KO, S], BF)              # 4 MB
            w_sb = persist.tile([128, KO, 768], BF)          # 1.5 MB

            nc.vector.memset(v_sb[:, :, :, 64:65], 1.0)

            # ---- loads ----
            hT_src = hiddenT.rearrange("(ko p) s -> p ko s", p=128)
            w_src = w_qkv.rearrange("(ko p) n -> p ko n", p=128)
            for ko in range(KO):
                q = nc.sync if ko % 2 == 0 else nc.scalar
                q.dma_start(hT[:, ko, :], hT_src[:, ko, :])
                nc.gpsimd.dma_start(w_sb[:, ko, :], w_src[:, ko, :])
                if ko == 0:
                    nc.sync.dma_start(bqk_sb, bqk[:, :])
                    nc.sync.dma_start(bv_sb, bv_rep[:, :])
                    nc.gpsimd.dma_start(dm_sb, diag_mask[:, :])
                    nc.gpsimd.dma_start(lm_sb, logmult[:, :, :])
                    nc.gpsimd.dma_start(pw_sb, projw[:, :, :])

            # ---- phase-1 building blocks ----
            _gq = [0]

            def qk_group(nt, sc, eng):
                # one [128, 512] output group, contraction-chained; the
                # PSUM->SBUF copy (with per-partition bias) on DVE/GPSIMD.
                _gq[0] += 1
                ps = mix.tile([128, 512], F32, tag="m", name=f"qk{_gq[0]}")
                for ko in range(KO):
                    nc.tensor.matmul(
                        ps,
                        w_sb[:, ko, 128 * nt:128 * nt + 128],
                        hT[:, ko, 512 * sc:512 * sc + 512],
                        start=(ko == 0), stop=(ko == KO - 1),
                    )
                eng.tensor_scalar_add(
                    qk_sb[:, nt, 512 * sc:512 * sc + 512],
                    ps,
                    bqk_sb[:, nt:nt + 1],
                )

            def v_tile(st):
                ps = mix.tile([128, 512], F32, tag="m", name=f"v{st}")
                for ko in range(KO):
                    nc.tensor.matmul(
                        ps[:, 0:256],
                        hT[:, ko, 128 * st:128 * st + 128],
                        w_sb[:, ko, 512:768],
                        start=(ko == 0), stop=(ko == KO - 1),
                    )
                nc.gpsimd.tensor_add(
                    out=v_sb[:, st, :, 0:64],
                    in0=ps[:, 0:256].rearrange("p (h d) -> p h d", d=64),
                    in1=bv_sb.rearrange("p (h d) -> p h d", d=64),
                )

            # ---- phase-2 building blocks ----
            av_tiles = {}

            def get_av(lh, b):
                if (lh, b) not in av_tiles:
                    pool = p2av if b < 2 else mix
                    tag = "av" if b < 2 else "m"
                    av_tiles[(lh, b)] = pool.tile(
                        [128, 512], F32, tag=tag, name=f"av{lh}{b}")
                return av_tiles[(lh, b)]

            def cproj(tau):
                for ec in range(2):
                    ps = mix.tile([128, 512], F32, tag="m", name=f"pr{tau}{ec}")
                    for j in range(2):
                        nc.tensor.matmul(
                            ps,
                            aoT_sb[:, tau, j, :],
                            pw_sb[:, j, 512 * ec:512 * ec + 512],
                            start=(j == 0), stop=(j == 1),
                        )
                    o_sb = p2o.tile([128, 512], BF, tag="o")
                    cpq = nc.vector if ec == 0 else nc.gpsimd
                    cpq.tensor_copy(o_sb, ps)
                    oq = nc.sync if ec == 0 else nc.gpsimd
                    oq.dma_start(
                        out[128 * tau:128 * tau + 128,
                            512 * ec:512 * ec + 512],
                        o_sb,
                    )

            def drain(lh, b):
                # all q-tiles of this batch fully accumulated: reciprocal
                # of the denominator column, per-partition scale into ao_sb.
                t0, t1 = BATCHES[b]
                nb = t1 - t0
                av = av_tiles.pop((lh, b))
                j, hp = lh // 2, lh % 2
                rec = p2rec.tile([128, 8], F32, tag="rec")
                den = av[:, 0:65 * nb].rearrange(
                    "p (n c) -> p n c", c=65)[:, :, 64:65]
                nc.vector.reciprocal(rec[:, 0:nb], den)
                for k in range(nb):
                    tau = t0 + k
                    nc.vector.tensor_scalar_mul(
                        ao_sb[:, tau, j, 64 * hp:64 * hp + 64],
                        av[:, 65 * k:65 * k + 64],
                        rec[:, k:k + 1],
                    )
                if hp == 1:
                    # both heads of pair j drained: transpose ao[q, hd] ->
                    # aoT[hd, q] on the DMA xbar; after the last pair,
                    # this q-tile's c_proj is fully unblocked.
                    for k in range(nb):
                        tau = t0 + k
                        nc.sync.dma_start_transpose(
                            aoT_sb[:, tau, j, :], ao_sb[:, tau, j, :])
                    if lh == 3:
                        for k in range(nb):
                            cproj(t0 + k)

            def tail(lh, t, q0, width, at_sb):
                # exp consumers: causal 0/1 mask on the diagonal block
                # (GPSIMD, all-SBUF) + flipped AV accumulation.
                if q0 == 128 * t:
                    nc.gpsimd.tensor_mul(
                        out=at_sb[:, 0:128], in0=at_sb[:, 0:128],
                        in1=dm_sb,
                    )
                v_aug = v_sb[:, t, lh, :]
                for tau in range(q0 // 128, (q0 + width) // 128):
                    b = 0 if tau < 7 else (1 if tau < 14 else 2)
                    av = get_av(lh, b)
                    col = 65 * (tau - BATCHES[b][0])
                    off = 128 * tau - q0
                    nc.tensor.matmul(
                        av[:, col:col + 65],
                        at_sb[:, off:off + 128],
                        v_aug,
                        start=(t == 0), stop=(t == tau),
                        skip_group_check=True,
                    )
                if q0 + width == 1024 and t == 6:
                    drain(lh, 0)
                elif q0 + width == 2048 and t == 13:
                    drain(lh, 1)
                elif q0 + width == 2048 and t == 15:
                    drain(lh, 2)

            pending = []

            def piece(lh, t, hf):
                q0 = max(128 * t, 1024 * hf)
                q1 = 1024 * (hf + 1)
                if q0 >= q1:
                    return
                width = q1 - q0
                bp = 64 * (lh % 2)
                q_nt = lh // 2
                k_nt = 2 + lh // 2
                lhsT_k = qk_sb[bp:bp + 64, k_nt, 128 * t:128 * t + 128]
                sc_ps = p2sc.tile([128, 1024], F32, tag="sc")
                off = 0
                while off < width:
                    w512 = min(512, width - off)
                    nc.tensor.matmul(
                        sc_ps[:, off:off + w512],
                        lhsT_k,
                        qk_sb[bp:bp + 64, q_nt, q0 + off:q0 + off + w512],
                        start=True, stop=True,
                    )
                    off += w512
                at_sb = p2at.tile([128, 1024], BF, tag="attnT")
                nc.scalar.activation(
                    at_sb[:, :width], sc_ps[:, :width],
                    mybir.ActivationFunctionType.Exp,
                    bias=lm_sb[:, lh, t:t + 1], scale=0.125,
                )
                pending.append((lh, t, q0, width, at_sb))
                if len(pending) > 4:
                    tail(*pending.pop(0))

            # ---- interleaved emission: program order is engine priority ----
            V = nc.vector
            P = nc.gpsimd
            qk_group(2, 0, V)                 # k01 cols 0:512
            qk_group(0, 0, V)                 # q01 cols 0:512
            qk_group(0, 1, V)                 # q01 cols 512:1024
            for t in range(4):
                piece(0, t, 0)
            qk_group(2, 1, V)
            for t in range(4, 8):
                piece(0, t, 0)
            qk_group(0, 2, V)
            qk_group(0, 3, V)
            qk_group(2, 2, V)
            qk_group(2, 3, V)
            for t in range(16):
                piece(0, t, 1)
            for st in range(8):
                v_tile(st)
            for t in range(8):
                piece(1, t, 0)
            for st in range(8, 16):
                v_tile(st)
            for t in range(16):
                piece(1, t, 1)
            qk_group(1, 0, V)                 # q23 cols 0:512
            qk_group(3, 0, P)                 # k23 cols 0:512
            qk_group(1, 1, V)
            qk_group(3, 1, P)
            for t in range(8):
                piece(2, t, 0)
            qk_group(1, 2, V)
            qk_group(1, 3, P)
            qk_group(3, 2, V)
            qk_group(3, 3, P)
            for t in range(16):
                piece(2, t, 1)
            for t in range(8):
                piece(3, t, 0)
            for t in range(16):
                piece(3, t, 1)
            for pc in pending:
                tail(*pc)
            pending.clear()
    return nc


_NC = None


def _get_nc():
    global _NC
    if _NC is None:
        _NC = build_program()
    return _NC


# ---------------------------------------------------------------- host prep

def make_in_maps(hidden_states, c_attn_w, c_attn_b, c_proj_w):
    import ml_dtypes
    bf16 = ml_dtypes.bfloat16

    first_end = S // 3
    second_end = 2 * S // 3
    pos = np.arange(S)
    regions = [pos < first_end,
               (pos >= first_end) & (pos < second_end),
               pos >= second_end]
    mult = np.ones((H, S), dtype=np.float64)
    for h, r in HEAD_REGION.items():
        mult[h] = 1.0 + (FOCUS - 1.0) * regions[r].astype(np.float64)
    logm = np.log(mult).astype(np.float32)  # [H, S]

    p = np.arange(128)[:, None]
    j = np.arange(128)[None, :]
    diag = (j >= p).astype(np.float32)  # 0/1 keep-mask, applied post-exp

    in_maps = []
    for c in range(NCORES):
        b, g = divmod(c, GROUPS)
        h0 = HPC * g
        cs = slice(256 * g, 256 * g + 256)
        w_qkv = np.concatenate(
            [c_attn_w[:, cs], c_attn_w[:, 1024:2048][:, cs],
             c_attn_w[:, 2048:3072][:, cs]], axis=1,
        )
        bqk = np.concatenate(
            [c_attn_b[cs], c_attn_b[1024:2048][cs]]
        ).reshape(4, 128).T.copy().astype(np.float32)
        bv = np.broadcast_to(
            c_attn_b[2048:3072][cs], (128, 256)
        ).astype(np.float32).copy()
        # pw2[p, j, e]: head pair j=(2j, 2j+1); p<64 -> head 2j row p,
        # p>=64 -> head 2j+1 row p-64  (matches aoT partition layout)
        pw = c_proj_w[64 * h0:64 * h0 + 256, :].reshape(2, 128, D)
        pw = np.ascontiguousarray(pw.transpose(1, 0, 2))
        lm = logm[h0:h0 + HPC].reshape(HPC, S // 128, 128)
        lm = np.ascontiguousarray(lm.transpose(2, 0, 1)).astype(np.float32)
        in_maps.append({
            "hiddenT": np.ascontiguousarray(hidden_states[b].T).astype(bf16),
            "w_qkv": np.ascontiguousarray(w_qkv).astype(bf16),
            "bqk": bqk,
            "bv_rep": bv,
            "projw": pw.astype(bf16),
            "diag_mask": diag.astype(bf16),
            "logmult": lm,
        })
    return in_maps


def run_cores(in_maps, trace=False, **kw):
    from concourse.bass_utils import run_bass_kernel_spmd
    nc = _get_nc()
    return run_bass_kernel_spmd(nc, in_maps, core_ids=list(range(NCORES)),
                                trace=trace, **kw)


def kernel(hidden_states, c_attn_w, c_attn_b, c_proj_w, c_proj_b):
    hidden_states = np.asarray(hidden_states, dtype=np.float32)
    c_attn_w = np.asarray(c_attn_w, dtype=np.float32)
    c_attn_b = np.asarray(c_attn_b, dtype=np.float32)
    c_proj_w = np.asarray(c_proj_w, dtype=np.float32)
    c_proj_b = np.asarray(c_proj_b, dtype=np.float32)

    in_maps = make_in_maps(hidden_states, c_attn_w, c_attn_b, c_proj_w)
    res = run_cores(in_maps)
    out = np.zeros((B, S, D), dtype=np.float32)
    for c in range(NCORES):
        out[c // GROUPS] += np.asarray(res.results[c]["out"], dtype=np.float32)
    out += c_proj_b[None, None, :]
    return out
